# revision 14
# baseline (speedup 1.0000x reference)
"""Trainium2 Bass kernel for an attention-LSTM caption decoder (show-attend-tell).

Data-parallel over batch: 64 examples -> 8 cores x 8 examples. Parameters are
SBUF-resident (bf16), the 51 decode steps are fully unrolled, and the fc
projection is deferred to one batched matmul at the end.

Host side: length-sort, embedding gather + projection of the embedding slice
of the LSTM input, bias folding, weight pre-scaling (sigmoid is evaluated as
0.5*(1+tanh(x/2)) on ACT; the resulting x2 on h/c is folded into pre-scaled
weights), final masking and output assembly.

Per-example (M=1) matmuls (e and ctx) use M=32 zero-padded stationaries with
tile_position=(0,32j), placing 4 examples per psum tile at partitions
{0,32,64,96}; lane-aligned strided ops + DMA gathers bridge back to the
contiguous [8,*] layout used by the LSTM block.
"""

import sys

for _p in ("/opt/trn_rl_repo",):
    if _p not in sys.path:
        sys.path.insert(0, _p)

import numpy as np
import ml_dtypes

import concourse.bass as bass
import concourse.bacc as bacc
import concourse.mybir as mybir
import concourse.tile as tile
from concourse.bass_utils import run_bass_kernel_spmd

F32 = mybir.dt.float32
BF16 = mybir.dt.bfloat16
F8 = mybir.dt.float8e4
AF = mybir.ActivationFunctionType
ALU = mybir.AluOpType

B, P, ENC = 64, 196, 2048
A, E, D, V, L = 512, 512, 512, 10000, 52
T_FULL = 51
NCORES = 8
BL = B // NCORES     # 8
PP = 256
NK_ENC = ENC // 128  # 16
NK_A = A // 128      # 4
NK_D = D // 128      # 4
MSLOTS = 512         # padded (t,ex) rows for fc (>= 51*8 = 408)

_nc_cache = {}


def _np_bf16(x):
    return np.ascontiguousarray(np.asarray(x).astype(ml_dtypes.bfloat16))


def _np_f32(x):
    return np.ascontiguousarray(np.asarray(x).astype(np.float32))


def build_nc(T=T_FULL):
    nc = bacc.Bacc("TRN2", target_bir_lowering=False, debug=False,
                   num_devices=NCORES)
    dram = {}

    def din(name, shape, dt):
        dram[name] = nc.dram_tensor(name, list(shape), dt, kind="ExternalInput")

    def dout(name, shape, dt):
        dram[name] = nc.dram_tensor(name, list(shape), dt, kind="ExternalOutput")

    din("encT", (BL, NK_ENC, 128, P), BF16)
    din("encN", (128, 2, BL, ENC), BF16)
    din("w_enc", (NK_ENC, 128, A), BF16)
    din("b_att1", (NK_A, 128, 1), F32)
    din("wfullp", (128, NK_A, BL, 32), BF16)    # att_W_full at col ex of block ex
    din("wdec", (128, NK_D * NK_A, 128), BF16)  # 0.5*att_W_dec tiles [k*4+m]
    din("wfb", (128, NK_D, ENC), BF16)          # 0.5*f_beta_W
    din("fbrow", (1, ENC), BF16)                 # f_beta_b row
    din("whh", (128, NK_D, 4 * D), BF16)        # 0.5*lstm_W_hh
    din("wihc", (128, NK_ENC // 2, 4 * D), BF16)  # 0.5*W_ih[E:] chunks 0-7 (resident)
    din("wihcS", (NK_ENC // 2, 128, 4 * D), BF16)  # chunks 8-15 (streamed per step)
    din("embp", (T, BL, 4 * D), BF16)
    din("i8", (BL, BL), BF16)
    din("identb", (128, 128), BF16)
    din("h0T", (128, NK_D, BL), BF16)           # 2*h0 transposed
    din("c20", (BL, D), F32)                    # 2*c0
    din("fcW", (NK_D, 128, V), BF16)            # 0.5*fc_W
    dout("preds", (4, 128, V), F32)
    dout("alphas", (T, BL, P), F32)

    with tile.TileContext(nc) as tc:
        with (
            tc.tile_pool(name="weights", bufs=1) as wpool,
        ):
            encN = wpool.tile([128, 2, BL, ENC], BF16)
            att1T = wpool.tile([128, BL, NK_A, P], BF16)
            wfullp = wpool.tile([128, NK_A, BL, 32], BF16)
            wdec = wpool.tile([128, NK_D * NK_A, 128], BF16)
            wfb = wpool.tile([128, NK_D, ENC], BF16)
            whh = wpool.tile([128, NK_D, 4 * D], BF16)
            wihc = wpool.tile([128, NK_ENC // 2, 4 * D], BF16)
            i8 = wpool.tile([BL, BL], BF16)
            identb = wpool.tile([128, 128], BF16)
            fbrow = wpool.tile([1, ENC], BF16)
            ones18 = wpool.tile([1, BL], BF16)
            b_att1 = wpool.tile([128, NK_A, 1], F32)
            h_hist = wpool.tile([128, NK_D, MSLOTS], BF16)
            h0T = wpool.tile([128, NK_D, BL], BF16)
            alpha8 = wpool.tile([BL, PP], BF16)
            alphaTp = wpool.tile([128, 2, BL, 32], BF16)

            nc.sync.dma_start(encN[:], dram["encN"][:])
            nc.sync.dma_start(wfullp[:], dram["wfullp"][:])
            nc.sync.dma_start(b_att1[:], dram["b_att1"][:].rearrange("k p o -> p k o"))
            for name, t_ in (("wdec", wdec), ("wfb", wfb), ("whh", whh),
                             ("wihc", wihc), ("i8", i8), ("identb", identb),
                             ("fbrow", fbrow), ("h0T", h0T)):
                nc.sync.dma_start(t_[:], dram[name][:])
            nc.vector.memset(ones18[:], 1.0)
            nc.vector.memset(h_hist[:, :, T * BL:], 0.0)
            nc.vector.memset(alpha8[:], 0.0)
            nc.vector.memset(alphaTp[:], 0.0)

            # ---------- att1 precompute ----------
            with (
                tc.tile_pool(name="wencp", bufs=1) as wencp,
                tc.tile_pool(name="encTq", bufs=1) as encTq,
                tc.tile_pool(name="psatt1", bufs=2, space="PSUM") as psa1,
            ):
                wenc = wencp.tile([128, NK_ENC, A], BF16)
                nc.sync.dma_start(wenc[:], dram["w_enc"][:].rearrange("k p a -> p k a"))
                for ex in range(BL):
                    encT = encTq.tile([128, NK_ENC, P], BF16, tag="encT")
                    nc.sync.dma_start(encT[:], dram["encT"][ex].rearrange("k p n -> p k n"))
                    for m in range(NK_A):
                        ps = psa1.tile([128, P], F32, tag="a1")
                        for k in range(NK_ENC):
                            nc.tensor.matmul(
                                ps[:], wenc[:, k, m * 128:(m + 1) * 128],
                                encT[:, k, :], start=(k == 0), stop=(k == NK_ENC - 1))
                        nc.scalar.activation(att1T[:, ex, m, :], ps[:],
                                             AF.Identity, bias=b_att1[:, m, :])

            # ---------- decode loop ----------
            loop_ctx = [
                tc.tile_pool(name="c2pool", bufs=2),
                tc.tile_pool(name="embq", bufs=1),
                tc.tile_pool(name="scr", bufs=2),
                tc.tile_pool(name="ptw", bufs=1),
                tc.tile_pool(name="xpool", bufs=2),
                tc.tile_pool(name="wstr", bufs=3),
                tc.tile_pool(name="psgates", bufs=1, space="PSUM"),
                tc.tile_pool(name="psctx", bufs=2, space="PSUM"),
                tc.tile_pool(name="psmisc", bufs=2, space="PSUM"),
            ]
            c2pool, embq, scr, ptw, xpool, wstr, psg, psc, psm = [
                c.__enter__() for c in loop_ctx]
            for t in range(T):
                def hT_(k, t=t):
                    if t == 0:
                        return h0T[:, k, :]
                    return h_hist[:, k, (t - 1) * BL:t * BL]

                if t == 0:
                    c2_prev = c2pool.tile([BL, D], F32, tag="c2")
                    nc.sync.dma_start(c2_prev[:], dram["c20"][:])
                else:
                    c2_prev = c2_state

                embp_t = embq.tile([BL, 4 * D], BF16, tag="embp")
                nc.sync.dma_start(embp_t[:], dram["embp"][t])
                wihcs = []
                for c in range(NK_ENC // 2):
                    wt = wstr.tile([128, 4 * D], BF16, tag="wstr")
                    eng = nc.sync if c % 2 == 0 else nc.scalar
                    eng.dma_start(wt[:], dram["wihcS"][c])
                    wihcs.append(wt)

                # att2_T = (0.5 W_dec).T @ h2  -> [128, NK_A, BL] f32 sbuf
                att2T = scr.tile([128, NK_A, BL], F32, tag="att2T")
                for m in range(NK_A):
                    ps = psm.tile([128, BL], F32, tag="m")
                    for k in range(NK_D):
                        nc.tensor.matmul(ps[:], wdec[:, k * NK_A + m, :], hT_(k),
                                         start=(k == 0), stop=(k == NK_D - 1))
                    nc.vector.tensor_copy(att2T[:, m, :], ps[:])

                # gatePre -> tgate = tanh(0.5*(h@f_beta + b)); also DMA-scatter
                # tgate into the strided group layout for the ctx STT.
                tgate = ptw.tile([BL, ENC], BF16, tag="tgate")
                for n in range(4):
                    ps = psm.tile([BL, 512], F32, tag="m")
                    sl = slice(n * 512, (n + 1) * 512)
                    for k in range(NK_D):
                        nc.tensor.matmul(ps[:], hT_(k), wfb[:, k, sl],
                                         start=(k == 0), stop=False)
                    nc.tensor.matmul(ps[:], ones18[:], fbrow[:, sl],
                                     start=False, stop=True)
                    nc.scalar.activation(tgate[:, sl], ps[:], AF.Tanh, scale=0.5)

                # attention: X = relu(att1 + att2); e accumulated into rows 0-7
                pe = psm.tile([32, 512], F32, tag="m")
                for ex in range(BL):
                    X = xpool.tile([128, NK_A * P], BF16, tag="X")
                    for c in range(NK_A):
                        nc.vector.tensor_scalar(
                            X[:, c * P:(c + 1) * P], att1T[:, ex, c, :],
                            att2T[:, c, ex:ex + 1], 0.0, ALU.add, ALU.max)
                    for c in range(NK_A):
                        nc.tensor.matmul(
                            pe[:, 0:P], wfullp[:, c, ex, :],
                            X[:, c * P:(c + 1) * P],
                            start=(ex == 0 and c == 0),
                            stop=(ex == BL - 1 and c == NK_A - 1))

                # softmax over rows 0-7
                negmax = ptw.tile([BL, 1], F32, tag="negmax")
                nc.vector.tensor_reduce(negmax[:], pe[0:BL, 0:P],
                                        mybir.AxisListType.X, ALU.max,
                                        negate=True)
                expo = ptw.tile([BL, P], F32, tag="expo")
                sume = ptw.tile([BL, 1], F32, tag="sume")
                nc.scalar.activation(expo[:], pe[0:BL, 0:P], AF.Exp,
                                     bias=negmax[:], accum_out=sume[:])
                rec = ptw.tile([BL, 1], F32, tag="rec")
                nc.vector.reciprocal(rec[:], sume[:])
                nc.vector.tensor_scalar(expo[:], expo[:], rec[:], None, ALU.mult)
                nc.sync.dma_start(dram["alphas"][t], expo[:])
                nc.vector.tensor_copy(alpha8[:, 0:P], expo[:])

                # transpose alpha8; scatter diagonally into alphaTp blocks
                for c in range(2):
                    pst = psm.tile([128, BL], BF16, tag="m")
                    nc.tensor.transpose(pst[:], alpha8[:, c * 128:(c + 1) * 128],
                                        identb[:BL, :BL])
                    nc.vector.tensor_copy(
                        alphaTp[:, c].rearrange("p e o -> p (e o)")[:, 0:256:33],
                        pst[:])

                # ctx accumulated into rows 0-7 per 512-chunk; xg2 = (tgate+1)*ctx
                xg8 = ptw.tile([BL, ENC], BF16, tag="xg8")
                for n in range(4):
                    ctx = psc.tile([32, 512], F32, tag="ctx")
                    nsl = slice(n * 512, (n + 1) * 512)
                    for ex in range(BL):
                        for c in range(2):
                            nc.tensor.matmul(
                                ctx[:], alphaTp[:, c, ex, :],
                                encN[:, c, ex, nsl],
                                start=(ex == 0 and c == 0),
                                stop=(ex == BL - 1 and c == 1))
                    nc.vector.scalar_tensor_tensor(
                        xg8[:, nsl], tgate[:, nsl], 1.0, ctx[0:BL, :],
                        ALU.add, ALU.mult)

                # xgT tiles + gates = ghh + emb + xg@wihc
                xgT = ptw.tile([128, NK_ENC, BL], BF16, tag="xgT")
                for c in range(NK_ENC):
                    pst = psm.tile([128, BL], BF16, tag="m")
                    nc.tensor.transpose(pst[:], xg8[:, c * 128:(c + 1) * 128],
                                        identb[:BL, :BL])
                    nc.vector.tensor_copy(xgT[:, c, :], pst[:])

                gates = psg.tile([BL, 4 * D], F32, tag="gates")
                for n in range(4):
                    sl = slice(n * 512, (n + 1) * 512)
                    for k in range(NK_D):
                        nc.tensor.matmul(gates[:, sl], hT_(k), whh[:, k, sl],
                                         start=(k == 0), stop=False)
                    nc.tensor.matmul(gates[:, sl], i8[:], embp_t[:, sl],
                                     start=False, stop=False)
                for c in range(NK_ENC):
                    w_ = (wihc[:, c, :] if c < NK_ENC // 2
                          else wihcs[c - NK_ENC // 2])
                    for n in range(4):
                        sl = slice(n * 512, (n + 1) * 512)
                        nc.tensor.matmul(gates[:, sl], xgT[:, c, :], w_[:, sl],
                                         start=False, stop=(c == NK_ENC - 1))

                # LSTM pointwise
                ti = ptw.tile([BL, D], BF16, tag="ti")
                tf = ptw.tile([BL, D], BF16, tag="tf")
                tg = ptw.tile([BL, D], BF16, tag="tg")
                to = ptw.tile([BL, D], BF16, tag="to")
                nc.scalar.activation(ti[:], gates[:, 0:512], AF.Tanh, scale=0.5)
                nc.scalar.activation(tf[:], gates[:, 512:1024], AF.Tanh, scale=0.5)
                nc.scalar.activation(tg[:], gates[:, 1024:1536], AF.Tanh)
                nc.scalar.activation(to[:], gates[:, 1536:2048], AF.Tanh, scale=0.5)

                Atl = ptw.tile([BL, D], F32, tag="Atl")
                Btl = ptw.tile([BL, D], F32, tag="Btl")
                nc.vector.scalar_tensor_tensor(Atl[:], tf[:], 1.0, c2_prev[:],
                                               ALU.add, ALU.mult)
                nc.vector.scalar_tensor_tensor(Btl[:], ti[:], 1.0, tg[:],
                                               ALU.add, ALU.mult)
                c2_state = c2pool.tile([BL, D], F32, tag="c2")
                nc.vector.scalar_tensor_tensor(c2_state[:], Atl[:], 0.5, Btl[:],
                                               ALU.mult, ALU.add)
                th = ptw.tile([BL, D], BF16, tag="th")
                nc.scalar.activation(th[:], c2_state[:], AF.Tanh, scale=0.5)
                h2 = ptw.tile([BL, D], BF16, tag="h2")
                nc.vector.scalar_tensor_tensor(h2[:], to[:], 1.0, th[:],
                                               ALU.add, ALU.mult)
                for k in range(NK_D):
                    pst = psm.tile([128, BL], BF16, tag="m")
                    nc.tensor.transpose(pst[:], h2[:, k * 128:(k + 1) * 128],
                                        identb[:BL, :BL])
                    nc.vector.tensor_copy(h_hist[:, k, t * BL:(t + 1) * BL], pst[:])

            for cm in reversed(loop_ctx):
                cm.__exit__(None, None, None)

            # ---------- fc ----------
            with (
                tc.tile_pool(name="fcq", bufs=8) as fcq,
                tc.tile_pool(name="fcout", bufs=4) as fcout,
                tc.tile_pool(name="psfc", bufs=4, space="PSUM") as psfc,
            ):
                for mt in range(4):
                    msl = slice(mt * 128, (mt + 1) * 128)
                    for nv in range(20):
                        n0, n1 = nv * 512, min(V, nv * 512 + 512)
                        ps = psfc.tile([128, 512], F32, tag="fc")
                        for k in range(NK_D):
                            fw = fcq.tile([128, 512], BF16, tag="fcw")
                            nc.sync.dma_start(fw[:, :n1 - n0], dram["fcW"][k, :, n0:n1])
                            nc.tensor.matmul(ps[:, :n1 - n0], h_hist[:, k, msl],
                                             fw[:, :n1 - n0],
                                             start=(k == 0), stop=(k == NK_D - 1))
                        ot = fcout.tile([128, 512], F32, tag="fco")
                        nc.scalar.copy(ot[:, :n1 - n0], ps[:, :n1 - n0])
                        nc.sync.dma_start(dram["preds"][mt, :, n0:n1], ot[:, :n1 - n0])

    nc.compile()
    return nc


def _prep_host(inputs):
    enc = _np_f32(inputs["encoder_out"])
    caps = np.asarray(inputs["encoded_captions"])
    caplen = np.asarray(inputs["caption_lengths"])
    g = {k: _np_f32(v) for k, v in inputs.items()
         if k not in ("encoder_out", "encoded_captions", "caption_lengths")}

    lengths = caplen[:, 0]
    sort_ind = np.argsort(-lengths, kind="stable")
    enc_s = enc[sort_ind]
    caps_s = caps[sort_ind]
    decode_lengths = lengths[sort_ind] - 1
    T = caps.shape[1] - 1

    emb = g["emb_table"][caps_s[:, :T]]
    embp = (emb.reshape(-1, E) @ g["lstm_W_ih"][:E]
            + g["lstm_b_ih"] + g["lstm_b_hh"]).reshape(B, T, 4 * D)
    mean_enc = enc_s.mean(axis=1)
    h0 = mean_enc @ g["init_h_W"] + g["init_h_b"]
    c0 = mean_enc @ g["init_c_W"] + g["init_c_b"]
    b_att1 = g["att_b_enc"] + g["att_b_dec"]

    wdec_t = np.zeros((128, NK_D * NK_A, 128), np.float32)
    for k in range(NK_D):
        for m in range(NK_A):
            wdec_t[:, k * NK_A + m, :] = 0.5 * g["att_W_dec"][
                k * 128:(k + 1) * 128, m * 128:(m + 1) * 128]
    wfullp = np.zeros((128, NK_A, BL, 32), np.float32)
    for c in range(NK_A):
        for ex in range(BL):
            wfullp[:, c, ex, ex] = g["att_W_full"][c * 128:(c + 1) * 128, 0]

    shared = {
        "w_enc": _np_bf16(g["att_W_enc"].reshape(NK_ENC, 128, A)),
        "b_att1": _np_f32(b_att1.reshape(NK_A, 128, 1)),
        "wfullp": _np_bf16(wfullp),
        "wdec": _np_bf16(wdec_t),
        "wfb": _np_bf16((0.5 * g["f_beta_W"]).reshape(NK_D, 128, ENC).transpose(1, 0, 2)),
        "fbrow": _np_bf16(g["f_beta_b"].reshape(1, ENC)),
        "whh": _np_bf16((0.5 * g["lstm_W_hh"]).reshape(NK_D, 128, 4 * D).transpose(1, 0, 2)),
        "wihc": _np_bf16((0.5 * g["lstm_W_ih"][E:E + 1024]).reshape(NK_ENC // 2, 128, 4 * D).transpose(1, 0, 2)),
        "wihcS": _np_bf16((0.5 * g["lstm_W_ih"][E + 1024:]).reshape(NK_ENC // 2, 128, 4 * D)),
        "i8": np.eye(BL, dtype=ml_dtypes.bfloat16),
        "identb": np.eye(128, dtype=ml_dtypes.bfloat16),
        "fcW": _np_bf16((0.5 * g["fc_W"]).reshape(NK_D, 128, V)),
    }

    in_maps = []
    for core in range(NCORES):
        sl = slice(core * BL, (core + 1) * BL)
        enc_c = enc_s[sl]
        encN = np.zeros((128, 2, BL, ENC), np.float32)
        encN[:, 0] = enc_c[:, :128].transpose(1, 0, 2)
        encN[:P - 128, 1] = enc_c[:, 128:].transpose(1, 0, 2)
        h0T_c = np.zeros((128, NK_D, BL), np.float32)
        for k in range(NK_D):
            h0T_c[:, k, :] = (2.0 * h0[sl, k * 128:(k + 1) * 128]).T
        m = dict(shared)
        m.update({
            "encT": _np_bf16(enc_c.transpose(0, 2, 1).reshape(BL, NK_ENC, 128, P)),
            "encN": _np_bf16(encN),
            "embp": _np_bf16(embp[sl].transpose(1, 0, 2)),
            "h0T": _np_bf16(h0T_c),
            "c20": _np_f32(2.0 * c0[sl]),
        })
        in_maps.append(m)

    host = {"sort_ind": sort_ind, "caps_s": caps_s,
            "decode_lengths": decode_lengths, "T": T, "fc_b": g["fc_b"]}
    return in_maps, host


def kernel(**inputs):
    in_maps, host = _prep_host(inputs)
    T = host["T"]
    if T not in _nc_cache:
        _nc_cache[T] = build_nc(T)
    nc = _nc_cache[T]

    res = run_bass_kernel_spmd(nc, in_maps, core_ids=list(range(NCORES)))

    preds = np.zeros((B, T, V), np.float32)
    alphas = np.zeros((B, T, P), np.float32)
    for core in range(NCORES):
        sl = slice(core * BL, (core + 1) * BL)
        pr = res.results[core]["preds"].reshape(4 * 128, V)[:T * BL]
        preds[sl] = pr.reshape(T, BL, V).transpose(1, 0, 2)
        alphas[sl] = res.results[core]["alphas"].transpose(1, 0, 2)

    preds += host["fc_b"][None, None, :]
    tmask = (host["decode_lengths"][:, None] > np.arange(T)[None, :])
    preds *= tmask[:, :, None]
    alphas *= tmask[:, :, None]

    return (preds,
            host["caps_s"].astype(np.int32),
            host["decode_lengths"].astype(np.int32),
            alphas,
            host["sort_ind"].astype(np.int32))
